# revision 21
# baseline (speedup 1.0000x reference)
"""Trainium2 Bass kernel for BiBo attention (GQA + per-head RMSNorm + RoPE +
SSMax scaling + causal attention + o_proj).

Sharding: tensor-parallel over the 4 KV-head groups x data-parallel over the
2 batch elements = 8 cores. Each core computes its 4 q-heads / 1 kv-head of
attention for one batch element plus its row-slice of o_proj; the host sums
the 4 partial o_proj outputs per batch element (row-parallel unshard).

Layout strategy (per core):
  - hidden^T [H, S] streamed from DRAM; projections produce q^T/k^T with the
    head dim on partitions so QK^T needs no transposes.
  - scores are computed transposed (scoresT[k, q]) so the PV matmul consumes
    exp(scoresT) directly; the softmax denominator is a ones-vector matmul
    (partition-dim sum on the PE), and no max-subtraction is needed because
    RMS-normed q/k bound |scores| <= sqrt(HD)*ssmax*log(S) ~ 10.
  - block-sparse causal skipping: mask blocks that are entirely <= -1e8 are
    skipped (their exp underflows to exactly 0 in fp32); all-zero blocks skip
    the mask add; others add the real mask values. The block plan is derived
    from the actual attention_mask at call time.
"""

import math

import numpy as np

B, S, H = 2, 2048, 2048
NH, NKV, HD = 16, 4, 128
EPS = 1e-6
NCORES = 8
TP = 4            # kv-head groups
QH = NH // NKV    # q heads per core
SC = 512          # q-tile / s-chunk width
NSC = S // SC     # 4
KT = 128          # k tile
NKT = S // KT     # 16
HC = 128          # h contraction chunk
NHC = H // HC     # 16
SKIP_THRESH = -1e8

_compiled_cache = {}
LAST_EXEC_NS = None
LAST_RESULTS = None


def _enable_ldw_opt():
    import os
    if not os.environ.get("BASS_LDW_OPT"):
        return
    from concourse import bass_utils as bu
    if getattr(bu.run_command, "_ldw_patched", False):
        return
    orig = bu.run_command

    def patched(argv, **kw):
        argv = ["--enable-ldw-opt=true" if a == "--enable-ldw-opt=false" else a
                for a in argv]
        return orig(argv, **kw)

    patched._ldw_patched = True
    bu.run_command = patched


def _build_program(plan, mask_counts):
    import concourse.mybir as mybir
    import concourse.tile as tile
    from concourse import bacc

    F32 = mybir.dt.float32
    F32R = mybir.dt.float32r
    MM = mybir.dt.bfloat16
    AF = mybir.ActivationFunctionType
    OP = mybir.AluOpType

    n_mask = sum(mask_counts)

    _enable_ldw_opt()
    nc = bacc.Bacc("TRN2", target_bir_lowering=False, debug=False,
                   num_devices=NCORES)
    hT = nc.dram_tensor("hT", [H, S], MM, kind="ExternalInput").ap()
    wqT = nc.dram_tensor("wqT", [H, QH * HD], MM, kind="ExternalInput").ap()
    wkT = nc.dram_tensor("wkT", [H, HD], MM, kind="ExternalInput").ap()
    wvT = nc.dram_tensor("wvT", [H, HD], MM, kind="ExternalInput").ap()
    woT = nc.dram_tensor("woT", [QH * HD, H], MM, kind="ExternalInput").ap()
    cosT = nc.dram_tensor("cosT", [HD, S], F32, kind="ExternalInput").ap()
    sinT = nc.dram_tensor("sinT", [HD, S], F32, kind="ExternalInput").ap()
    sgn = nc.dram_tensor("sgn", [HD, 1], F32, kind="ExternalInput").ap()
    qsc = nc.dram_tensor("qsc", [1, QH], F32, kind="ExternalInput").ap()
    qsb = nc.dram_tensor("qsb", [1, QH], F32, kind="ExternalInput").ap()
    iwq = nc.dram_tensor("iwq", [HD, 1], MM, kind="ExternalInput").ap()
    iwk = nc.dram_tensor("iwk", [HD, 1], MM, kind="ExternalInput").ap()
    if n_mask:
        mblk = nc.dram_tensor("mblk", [n_mask, KT, SC], F32,
                              kind="ExternalInput").ap()
        mtri = nc.dram_tensor("mtri", [KT, KT], MM, kind="ExternalInput").ap()
        mhot = nc.dram_tensor("mhot", [n_mask, KT, SC], MM,
                              kind="ExternalInput").ap()
    out = nc.dram_tensor("out", [S, H], F32, kind="ExternalOutput").ap()

    with tile.TileContext(nc) as tc:
        _emit(nc, tc, locals(), plan, mask_counts, MM, F32, F32R, AF, OP)
    nc.compile()
    return nc


def _emit(nc, tc, T, plan, mask_counts, MM, F32, F32R, AF, OP):
    from contextlib import ExitStack

    hT, wqT, wkT, wvT, woT = T["hT"], T["wqT"], T["wkT"], T["wvT"], T["woT"]
    cosT, sinT, sgn = T["cosT"], T["sinT"], T["sgn"]
    qsc, qsb = T["qsc"], T["qsb"]
    iwq, iwk, out = T["iwq"], T["iwk"], T["out"]
    mblk = T.get("mblk")
    mtri = T.get("mtri")
    mhot = T.get("mhot")

    ctx = ExitStack()
    with ctx:
        const = ctx.enter_context(tc.tile_pool(name="const", bufs=1))
        wpool = ctx.enter_context(tc.tile_pool(name="w", bufs=1))
        persist = ctx.enter_context(tc.tile_pool(name="persist", bufs=1))
        hpool = ctx.enter_context(tc.tile_pool(name="h", bufs=18))
        mpool = ctx.enter_context(tc.tile_pool(name="m", bufs=6))
        spool = ctx.enter_context(tc.tile_pool(name="s", bufs=2))
        epool = ctx.enter_context(tc.tile_pool(name="e", bufs=3))
        atpool = ctx.enter_context(tc.tile_pool(name="at", bufs=8))
        opool_sb = ctx.enter_context(tc.tile_pool(name="osb", bufs=3))
        ps_mm = ctx.enter_context(tc.tile_pool(name="psmm", bufs=2, space="PSUM"))
        ps_v = ctx.enter_context(tc.tile_pool(name="psv", bufs=1, space="PSUM"))
        ps_pv = ctx.enter_context(tc.tile_pool(name="pspv", bufs=2, space="PSUM"))
        ps_es = ctx.enter_context(tc.tile_pool(name="pses", bufs=1, space="PSUM"))
        ps_o = ctx.enter_context(tc.tile_pool(name="pso", bufs=2, space="PSUM"))

        # ---- persistent tiles (loads emitted by the driver below) -------
        wq_t = wpool.tile([128, NHC * QH * HD], MM, tag="wq")
        wk_t = wpool.tile([128, NHC * HD], MM, tag="wk")
        wv_t = wpool.tile([128, NHC * HD], MM, tag="wv")
        wo_t = wpool.tile([128, QH * H], MM, tag="wo")
        cos_t = wpool.tile([128, S], F32, tag="cos")
        sin_t = wpool.tile([128, S], F32, tag="sin")
        sgn_t = const.tile([128, 1], F32, tag="sgn")
        qsc_t = const.tile([1, QH], F32, tag="qsc")
        qsb_t = const.tile([1, QH], F32, tag="qsb")
        iwq_t = const.tile([128, 1], MM, tag="iwq")
        iwk_t = const.tile([128, 1], MM, tag="iwk")
        ones_t = const.tile([128, 1], MM, tag="ones")
        eps_t = const.tile([1, 1], F32, tag="eps")
        tri_t = (const.tile([128, KT], MM, tag="tri", name="tri")
                 if mtri is not None else None)
        khat = persist.tile([128, S], MM, tag="khat")
        rstdk = persist.tile([128, NKT], F32, tag="rstdk")
        v_sb = persist.tile([128, S], MM, tag="v")
        qhat = [persist.tile([128, S], MM, name=f"qhat{i}", tag=f"qhat{i}")
                for i in range(QH)]

        def load_early():
            for c in range(NHC):
                nc.sync.dma_start(wq_t[:, c * QH * HD:(c + 1) * QH * HD],
                                  wqT[c * HC:(c + 1) * HC, :])
                nc.sync.dma_start(wk_t[:, c * HD:(c + 1) * HD],
                                  wkT[c * HC:(c + 1) * HC, :])
                nc.sync.dma_start(wv_t[:, c * HD:(c + 1) * HD],
                                  wvT[c * HC:(c + 1) * HC, :])
            nc.sync.dma_start(sgn_t[:], sgn[:])
            nc.sync.dma_start(qsc_t[:], qsc[:])
            nc.sync.dma_start(qsb_t[:], qsb[:])
            nc.sync.dma_start(iwq_t[:], iwq[:])
            nc.sync.dma_start(iwk_t[:], iwk[:])
            nc.vector.memset(ones_t[:], 1.0)
            nc.vector.memset(eps_t[:], EPS)
            if mtri is not None:
                nc.sync.dma_start(tri_t[:], mtri[:])
            nc.sync.dma_start(cos_t[:], cosT[:])
            nc.sync.dma_start(sin_t[:], sinT[:])

        def load_wo():
            for f in range(QH):
                nc.sync.dma_start(wo_t[:, f * H:(f + 1) * H],
                                  woT[f * HD:(f + 1) * HD, :])

        # norm+rope staged: s1 (right after the proj matmuls) does the
        # rotate-copy, square, and the cos-product -- the last PSUM read, so
        # the proj bank frees early; the var matmul (s2) trails by one
        # projection group; s3 finishes rstd + rope off the PSUM path.
        def norm_s1(pp, sc):
            sh = spool.tile([128, SC], F32, tag="sh", name="sh")
            nc.vector.tensor_copy(sh[0:64, :], pp[64:128, :])
            nc.vector.tensor_copy(sh[64:128, :], pp[0:64, :])
            # var from the rotated copy (rotation-invariant; iw_t rows are
            # pre-rotated host-side to match)
            sq = spool.tile([128, SC], MM, tag="sq", name="sq")
            nc.vector.tensor_mul(sq[:], sh[:], sh[:])
            uu = spool.tile([128, SC], F32, tag="uu", name="uu")
            nc.vector.tensor_mul(uu[:], pp[:], cos_t[:, sc * SC:(sc + 1) * SC])
            return sh, sq, uu

        def norm_s2(sq, iw_t):
            var = ps_mm.tile([1, SC], F32, tag="mm", name="var")
            nc.tensor.matmul(var[:], iw_t[:], sq[:], start=True, stop=True)
            return var

        def norm_s3(sh, uu, var, sc, hd, hat_dst):
            # sd = sqrt(var + eps)/qc via folded scale/bias; rs = qc/rms
            sd = spool.tile([1, SC], F32, tag="sd", name="sd")
            if hd is None:
                nc.scalar.activation(sd[:], var[:], AF.Sqrt, bias=eps_t[:])
            else:
                nc.scalar.activation(sd[:], var[:], AF.Sqrt,
                                     bias=qsb_t[:, hd:hd + 1],
                                     scale=qsc_t[:, hd:hd + 1])
            rs = spool.tile([1, SC], F32, tag="rs", name="rs")
            nc.vector.reciprocal_approx_fast(rs[:], sd[:])
            tt = spool.tile([128, SC], F32, tag="tt", name="tt")
            nc.vector.scalar_tensor_tensor(
                tt[:], sh[:], sgn_t[:], sin_t[:, sc * SC:(sc + 1) * SC],
                op0=OP.mult, op1=OP.mult)
            if hd is None:
                # k side: rope only; rms-normalization is folded into the
                # exp scale later via the transposed rstd store
                nc.gpsimd.tensor_add(hat_dst, tt[:], uu[:])
                for j in range(4):
                    nc.sync.dma_start(
                        rstdk[:, sc * 4 + j:sc * 4 + j + 1],
                        rs[0:1, j * 128:(j + 1) * 128])
            else:
                bb = spool.tile([128, SC], F32, tag="bb", name="bb")
                nc.gpsimd.partition_broadcast(bb[:], rs[:], 128)
                nc.gpsimd.tensor_add(tt[:], tt[:], uu[:])
                nc.vector.tensor_mul(hat_dst, tt[:], bb[:])

        # ---- projections, per s-chunk -----------------------------------
        def hts_load(sc):
            hts = []
            for c in range(NHC):
                t = hpool.tile([128, SC], MM, tag="ht", name="ht")
                nc.sync.dma_start(t[:], hT[c * HC:(c + 1) * HC,
                                           sc * SC:(sc + 1) * SC])
                hts.append(t)
            return hts

        def proj_chunk(sc, hts):
            # five projections (k, q0..q3), staged so each var matmul is
            # emitted after the NEXT projection's matmul group
            specs = [(iwk_t, None, khat)] + [
                (iwq_t, hd, qhat[hd]) for hd in range(QH)]
            state = []  # (sh, sq, uu, spec)

            def do_mm(idx):
                pp = ps_mm.tile([128, SC], F32, tag="mm", name="pp")
                for c in range(NHC):
                    if idx == 0:
                        w_sl = wk_t[:, c * HD:(c + 1) * HD]
                    else:
                        base = c * QH * HD + (idx - 1) * HD
                        w_sl = wq_t[:, base:base + HD]
                    nc.tensor.matmul(pp[:], w_sl, hts[c][:],
                                     start=(c == 0), stop=(c == NHC - 1))
                sh, sq, uu = norm_s1(pp, sc)
                state.append((sh, sq, uu, specs[idx]))

            def finish_one():
                sh, sq, uu, (iw_t, hd, dst) = state.pop(0)
                var = norm_s2(sq, iw_t)
                norm_s3(sh, uu, var, sc, hd, dst[:, sc * SC:(sc + 1) * SC])

            do_mm(0)
            for idx in range(1, 5):
                do_mm(idx)
                finish_one()
            finish_one()
            # v-proj: natural [s, d] layout, N=128 matmuls
            for ss in range(4):
                vp = ps_v.tile([128, HD], F32, tag="v", name="vp")
                for c in range(NHC):
                    nc.tensor.matmul(vp[:], hts[c][:, ss * 128:(ss + 1) * 128],
                                     wv_t[:, c * HD:(c + 1) * HD],
                                     start=(c == 0), stop=(c == NHC - 1))
                col = (sc * 4 + ss) * 128
                nc.vector.tensor_copy(v_sb[:, col:col + 128], vp[:])

        # ---- attention + o_proj, per q-tile ------------------------------
        mask_starts = [sum(mask_counts[:i]) for i in range(NSC)]

        def attn_qtile(qi):
            mask_idx = mask_starts[qi]
            kts = [kt for kt in range(NKT) if plan[qi][kt] != "skip"]
            # load this q-tile's mask blocks (shared across heads)
            mtiles = {}
            for kt in kts:
                if plan[qi][kt] == "step":
                    mt = mpool.tile([128, SC], MM, tag="maskh", name="mh")
                    nc.sync.dma_start(mt[:], mhot[mask_idx])
                    mtiles[kt] = ("step", mt)
                    mask_idx += 1
                elif plan[qi][kt] == "mask":
                    mt = mpool.tile([128, SC], F32, tag="mask", name="mk")
                    nc.sync.dma_start(mt[:], mblk[mask_idx])
                    mtiles[kt] = ("mask", mt)
                    mask_idx += 1
            ats = []
            for hd in range(QH):
                qsl = qhat[hd][:, qi * SC:(qi + 1) * SC]
                pv = ps_pv.tile([128, SC], F32, tag="pv")
                es = ps_es.tile([1, SC], F32, tag="es")
                sts = {}
                # pipeline QK^T one k-tile ahead of exp/PV
                for j, kt in enumerate(kts):
                    st = ps_mm.tile([128, SC], F32, tag="mm")
                    kind, mt = mtiles.get(kt, (None, None))
                    nc.tensor.matmul(st[:], khat[:, kt * 128:(kt + 1) * 128],
                                     qsl, start=True, stop=(kind != "step"))
                    if kind == "step":
                        nc.tensor.matmul(st[:], tri_t[:], mt[:],
                                         start=False, stop=True)
                    elif kind == "mask":
                        nc.vector.tensor_add(st[:], st[:], mt[:])
                    sts[j] = st
                    if j >= 1:
                        _attn_tail(nc, j - 1, kts, sts, es, pv, v_sb,
                                   ones_t, MM, AF, epool, rstdk)
                _attn_tail(nc, len(kts) - 1, kts, sts, es, pv, v_sb,
                           ones_t, MM, AF, epool, rstdk)
                rs = spool.tile([1, SC], F32, tag="ars")
                nc.vector.reciprocal_approx_fast(rs[:], es[:])
                bb = spool.tile([128, SC], F32, tag="abb")
                nc.gpsimd.partition_broadcast(bb[:], rs[:], 128)
                at = atpool.tile([128, SC], MM, tag="at")
                nc.vector.tensor_mul(at[:], pv[:], bb[:])
                ats.append(at)
            # o_proj for this q-tile
            for ss in range(4):
                for ho in range(4):
                    op_t = ps_o.tile([128, SC], F32, tag="o")
                    for hd in range(QH):
                        nc.tensor.matmul(
                            op_t[:],
                            ats[hd][:, ss * 128:(ss + 1) * 128],
                            wo_t[:, hd * H + ho * SC:hd * H + (ho + 1) * SC],
                            start=(hd == 0), stop=(hd == QH - 1))
                    ob = opool_sb.tile([128, SC], F32, tag="osb")
                    nc.vector.tensor_copy(ob[:], op_t[:])
                    nc.sync.dma_start(
                        out[qi * SC + ss * 128:qi * SC + (ss + 1) * 128,
                            ho * SC:(ho + 1) * SC],
                        ob[:])

        # ---- driver: software-pipelined phase order ----------------------
        hts0 = hts_load(0)
        load_early()
        proj_chunk(0, hts0)
        hts1 = hts_load(1)
        proj_chunk(1, hts1)
        load_wo()
        attn_qtile(0)
        hts2 = hts_load(2)
        proj_chunk(2, hts2)
        attn_qtile(1)
        hts3 = hts_load(3)
        proj_chunk(3, hts3)
        attn_qtile(2)
        attn_qtile(3)


def _attn_tail(nc, j, kts, sts, es, pv, v_sb, ones_t, MM, AF, epool, rstdk):
    """exp + PV + PSUM-accumulated denominator for pipelined k-tile j.

    The k-side RMS normalization is applied here as the exp scale
    (per-partition rstd_k column)."""
    kt = kts[j]
    st = sts.pop(j)
    ex = epool.tile([128, SC], MM, tag="ex", name="ex")
    nc.scalar.activation(ex[:], st[:], AF.Exp, scale=rstdk[:, kt:kt + 1])
    last = j == len(kts) - 1
    nc.tensor.matmul(pv[:], v_sb[:, kt * 128:(kt + 1) * 128], ex[:],
                     start=(j == 0), stop=last)
    nc.tensor.matmul(es[:], ones_t[:], ex[:], start=(j == 0), stop=last)


def _is_step(blk):
    """True if every batch/column is 0 for k < f and exactly -1e9 for k >= f."""
    isneg = blk == np.float32(-1e9)
    iszero = blk == 0.0
    if not (isneg | iszero).all():
        return False
    # per (b, q): suffix property along k
    f = isneg.argmax(axis=-1) + 0  # first masked k (0 if none masked)
    any_neg = isneg.any(axis=-1)
    kk = np.arange(blk.shape[-1])
    want = np.where(any_neg[..., None], kk[None, None] >= f[..., None], False)
    return bool((isneg == want).all())


def _mask_plan(mask):
    """Classify [qi][kt] blocks of the (q,k) mask, unified across batch."""
    plan = []
    for qi in range(NSC):
        row = []
        for kt in range(NKT):
            blk = mask[:, 0, qi * SC:(qi + 1) * SC, kt * KT:(kt + 1) * KT]
            if (blk <= SKIP_THRESH).all():
                row.append("skip")
            elif (blk == 0.0).all():
                row.append("zero")
            elif _is_step(blk):
                row.append("step")
            else:
                row.append("mask")
        # guard: a q-tile with no included block would divide by zero
        if all(s == "skip" for s in row):
            row[0] = "mask"
        plan.append(row)
    return plan


def kernel(hidden_states, cos, sin, attention_mask, wq, wk, wv, wo,
           q_norm_w, k_norm_w, ssmax_scale):
    global LAST_EXEC_NS
    import os
    import ml_dtypes
    from concourse.bass_utils import run_bass_kernel_spmd

    f32 = np.float32
    hidden_states = np.asarray(hidden_states, f32)
    cos = np.asarray(cos, f32)
    sin = np.asarray(sin, f32)
    attention_mask = np.asarray(attention_mask, f32)
    wq = np.asarray(wq, f32)
    wk = np.asarray(wk, f32)
    wv = np.asarray(wv, f32)
    wo = np.asarray(wo, f32)
    q_norm_w = np.asarray(q_norm_w, f32)
    k_norm_w = np.asarray(k_norm_w, f32)
    ssmax = np.asarray(ssmax_scale, f32).reshape(NH)

    plan = _mask_plan(attention_mask)
    mask_counts = [sum(1 for s in row if s in ("mask", "step")) for row in plan]
    key = (tuple(tuple(r) for r in plan),)
    if key not in _compiled_cache:
        _compiled_cache[key] = _build_program(plan, mask_counts)
    nc = _compiled_cache[key]

    bf16 = ml_dtypes.bfloat16
    qw = np.tile(q_norm_w, QH)
    sgn_np = np.concatenate([-np.ones(64, f32), np.ones(64, f32)])[:, None]
    iwq_np = np.roll(1.0 / (HD * q_norm_w ** 2), -64).astype(bf16)[:, None]
    iwk_np = np.roll(1.0 / (HD * k_norm_w ** 2), -64).astype(bf16)[:, None]
    cosT_np = np.ascontiguousarray(cos.T)
    sinT_np = np.ascontiguousarray(sin.T)

    in_maps = []
    for core in range(NCORES):
        b, g = divmod(core, TP)
        hTm = np.ascontiguousarray(hidden_states[b].T).astype(bf16)
        wq_s = wq[g * QH * HD:(g + 1) * QH * HD] * qw[:, None]
        wk_s = wk[g * HD:(g + 1) * HD] * k_norm_w[:, None]
        wv_s = wv[g * HD:(g + 1) * HD]
        wo_s = wo[:, g * QH * HD:(g + 1) * QH * HD]
        qcv = np.array([ssmax[g * QH + i] * math.log(S) / math.sqrt(HD)
                        for i in range(QH)], f32)
        qsc_np = (1.0 / qcv ** 2)[None, :].astype(f32)
        qsb_np = (EPS / qcv ** 2)[None, :].astype(f32)
        m = {
            "hT": hTm,
            "wqT": np.ascontiguousarray(wq_s.T).astype(bf16),
            "wkT": np.ascontiguousarray(wk_s.T).astype(bf16),
            "wvT": np.ascontiguousarray(wv_s.T).astype(bf16),
            "woT": np.ascontiguousarray(wo_s.T).astype(bf16),
            "cosT": cosT_np, "sinT": sinT_np, "sgn": sgn_np,
            "qsc": qsc_np, "qsb": qsb_np, "iwq": iwq_np, "iwk": iwk_np,
        }
        n_mask = sum(mask_counts)
        if n_mask:
            blocks = np.zeros((n_mask, KT, SC), f32)
            hots = np.zeros((n_mask, KT, SC), f32)
            i = 0
            for qi in range(NSC):
                for kt in range(NKT):
                    kind = plan[qi][kt]
                    if kind not in ("mask", "step"):
                        continue
                    blkT = attention_mask[
                        b, 0, qi * SC:(qi + 1) * SC,
                        kt * KT:(kt + 1) * KT].T
                    if kind == "mask":
                        blocks[i] = blkT
                    else:
                        isneg = blkT == np.float32(-1e9)
                        f = isneg.argmax(axis=0)
                        anyneg = isneg.any(axis=0)
                        qsel = np.nonzero(anyneg)[0]
                        hots[i][f[qsel], qsel] = 1.0
                    i += 1
            m["mblk"] = blocks
            m["mhot"] = hots.astype(bf16)
            # tri[r, k] = -1e9 * [k >= r]; lhsT layout [r(part), k(free)]
            tri = (-1e9 * (np.arange(KT)[None, :] >= np.arange(KT)[:, None]))
            m["mtri"] = np.ascontiguousarray(tri).astype(bf16)
        in_maps.append(m)

    trace = bool(int(os.environ.get("BASS_KERNEL_TRACE", "0")))
    res = run_bass_kernel_spmd(nc, in_maps, list(range(NCORES)), trace=trace)
    LAST_EXEC_NS = res.exec_time_ns
    globals()["LAST_RESULTS"] = res

    final = np.zeros((B, S, H), f32)
    for core in range(NCORES):
        b = core // TP
        final[b] += res.results[core]["out"]
    return final


# revision 22
# speedup vs baseline: 1.2798x; 1.2798x over previous
"""Trainium2 Bass kernel for BiBo attention (GQA + per-head RMSNorm + RoPE +
SSMax scaling + causal attention + o_proj).

Sharding: tensor-parallel over the 4 KV-head groups x data-parallel over the
2 batch elements = 8 cores. Each core computes its 4 q-heads / 1 kv-head of
attention for one batch element plus its row-slice of o_proj; the host sums
the 4 partial o_proj outputs per batch element (row-parallel unshard).

Layout strategy (per core):
  - hidden^T [H, S] streamed from DRAM; projections produce q^T/k^T with the
    head dim on partitions so QK^T needs no transposes.
  - scores are computed transposed (scoresT[k, q]) so the PV matmul consumes
    exp(scoresT) directly; the softmax denominator is a ones-vector matmul
    (partition-dim sum on the PE), and no max-subtraction is needed because
    RMS-normed q/k bound |scores| <= sqrt(HD)*ssmax*log(S) ~ 10.
  - block-sparse causal skipping: mask blocks that are entirely <= -1e8 are
    skipped (their exp underflows to exactly 0 in fp32); all-zero blocks skip
    the mask add; others add the real mask values. The block plan is derived
    from the actual attention_mask at call time.
"""

import math

import numpy as np

B, S, H = 2, 2048, 2048
NH, NKV, HD = 16, 4, 128
EPS = 1e-6
NCORES = 8
TP = 4            # kv-head groups
QH = NH // NKV    # q heads per core
SC = 512          # q-tile / s-chunk width
NSC = S // SC     # 4
KT = 128          # k tile
NKT = S // KT     # 16
HC = 128          # h contraction chunk
NHC = H // HC     # 16
SKIP_THRESH = -1e8

_compiled_cache = {}
LAST_EXEC_NS = None
LAST_RESULTS = None


def _enable_ldw_opt():
    import os
    if not os.environ.get("BASS_LDW_OPT"):
        return
    from concourse import bass_utils as bu
    if getattr(bu.run_command, "_ldw_patched", False):
        return
    orig = bu.run_command

    def patched(argv, **kw):
        argv = ["--enable-ldw-opt=true" if a == "--enable-ldw-opt=false" else a
                for a in argv]
        return orig(argv, **kw)

    patched._ldw_patched = True
    bu.run_command = patched


def _build_program(plan, mask_counts):
    import concourse.mybir as mybir
    import concourse.tile as tile
    from concourse import bacc

    F32 = mybir.dt.float32
    F32R = mybir.dt.float32r
    MM = mybir.dt.bfloat16
    AF = mybir.ActivationFunctionType
    OP = mybir.AluOpType

    n_mask = sum(mask_counts)

    _enable_ldw_opt()
    nc = bacc.Bacc("TRN2", target_bir_lowering=False, debug=False,
                   num_devices=NCORES)
    hT = nc.dram_tensor("hT", [H, S], MM, kind="ExternalInput").ap()
    wqT = nc.dram_tensor("wqT", [H, QH * HD], MM, kind="ExternalInput").ap()
    wkT = nc.dram_tensor("wkT", [H, HD], MM, kind="ExternalInput").ap()
    wvT = nc.dram_tensor("wvT", [H, HD], MM, kind="ExternalInput").ap()
    woT = nc.dram_tensor("woT", [QH * HD, H], MM, kind="ExternalInput").ap()
    cosT = nc.dram_tensor("cosT", [HD, S], F32, kind="ExternalInput").ap()
    sinT = nc.dram_tensor("sinT", [HD, S], F32, kind="ExternalInput").ap()
    sgn = nc.dram_tensor("sgn", [HD, 1], F32, kind="ExternalInput").ap()
    qsc = nc.dram_tensor("qsc", [1, QH], F32, kind="ExternalInput").ap()
    qsb = nc.dram_tensor("qsb", [1, QH], F32, kind="ExternalInput").ap()
    iwq = nc.dram_tensor("iwq", [HD, 1], MM, kind="ExternalInput").ap()
    iwk = nc.dram_tensor("iwk", [HD, 1], MM, kind="ExternalInput").ap()
    if n_mask:
        mblk = nc.dram_tensor("mblk", [n_mask, KT, SC], F32,
                              kind="ExternalInput").ap()
        mtri = nc.dram_tensor("mtri", [KT, KT], MM, kind="ExternalInput").ap()
        mhot = nc.dram_tensor("mhot", [n_mask, KT, SC], MM,
                              kind="ExternalInput").ap()
    out = nc.dram_tensor("out", [S, H], F32, kind="ExternalOutput").ap()

    with tile.TileContext(nc) as tc:
        _emit(nc, tc, locals(), plan, mask_counts, MM, F32, F32R, AF, OP)
    nc.compile()
    return nc


def _emit(nc, tc, T, plan, mask_counts, MM, F32, F32R, AF, OP):
    from contextlib import ExitStack

    hT, wqT, wkT, wvT, woT = T["hT"], T["wqT"], T["wkT"], T["wvT"], T["woT"]
    cosT, sinT, sgn = T["cosT"], T["sinT"], T["sgn"]
    qsc, qsb = T["qsc"], T["qsb"]
    iwq, iwk, out = T["iwq"], T["iwk"], T["out"]
    mblk = T.get("mblk")
    mtri = T.get("mtri")
    mhot = T.get("mhot")

    ctx = ExitStack()
    with ctx:
        const = ctx.enter_context(tc.tile_pool(name="const", bufs=1))
        wpool = ctx.enter_context(tc.tile_pool(name="w", bufs=1))
        persist = ctx.enter_context(tc.tile_pool(name="persist", bufs=1))
        hpool = ctx.enter_context(tc.tile_pool(name="h", bufs=18))
        mpool = ctx.enter_context(tc.tile_pool(name="m", bufs=6))
        spool = ctx.enter_context(tc.tile_pool(name="s", bufs=2))
        epool = ctx.enter_context(tc.tile_pool(name="e", bufs=3))
        atpool = ctx.enter_context(tc.tile_pool(name="at", bufs=8))
        opool_sb = ctx.enter_context(tc.tile_pool(name="osb", bufs=3))
        ps_mm = ctx.enter_context(tc.tile_pool(name="psmm", bufs=2, space="PSUM"))
        ps_v = ctx.enter_context(tc.tile_pool(name="psv", bufs=1, space="PSUM"))
        ps_pv = ctx.enter_context(tc.tile_pool(name="pspv", bufs=2, space="PSUM"))
        ps_es = ctx.enter_context(tc.tile_pool(name="pses", bufs=1, space="PSUM"))
        ps_o = ctx.enter_context(tc.tile_pool(name="pso", bufs=2, space="PSUM"))

        # ---- persistent tiles (loads emitted by the driver below) -------
        wq_t = wpool.tile([128, NHC * QH * HD], MM, tag="wq")
        wk_t = wpool.tile([128, NHC * HD], MM, tag="wk")
        wv_t = wpool.tile([128, NHC * HD], MM, tag="wv")
        wo_t = wpool.tile([128, QH * H], MM, tag="wo")
        cos_t = wpool.tile([128, S], F32, tag="cos")
        sin_t = wpool.tile([128, S], F32, tag="sin")
        sgn_t = const.tile([128, 1], F32, tag="sgn")
        qsc_t = const.tile([1, QH], F32, tag="qsc")
        qsb_t = const.tile([1, QH], F32, tag="qsb")
        iwq_t = const.tile([128, 1], MM, tag="iwq")
        iwk_t = const.tile([128, 1], MM, tag="iwk")
        ones_t = const.tile([128, 1], MM, tag="ones")
        eps_t = const.tile([1, 1], F32, tag="eps")
        tri_t = (const.tile([128, KT], MM, tag="tri", name="tri")
                 if mtri is not None else None)
        khat = persist.tile([128, S], MM, tag="khat")
        v_sb = persist.tile([128, S], MM, tag="v")
        qhat = [persist.tile([128, S], MM, name=f"qhat{i}", tag=f"qhat{i}")
                for i in range(QH)]

        def load_early():
            for c in range(NHC):
                nc.sync.dma_start(wq_t[:, c * QH * HD:(c + 1) * QH * HD],
                                  wqT[c * HC:(c + 1) * HC, :])
                nc.sync.dma_start(wk_t[:, c * HD:(c + 1) * HD],
                                  wkT[c * HC:(c + 1) * HC, :])
                nc.sync.dma_start(wv_t[:, c * HD:(c + 1) * HD],
                                  wvT[c * HC:(c + 1) * HC, :])
            nc.sync.dma_start(sgn_t[:], sgn[:])
            nc.sync.dma_start(qsc_t[:], qsc[:])
            nc.sync.dma_start(qsb_t[:], qsb[:])
            nc.sync.dma_start(iwq_t[:], iwq[:])
            nc.sync.dma_start(iwk_t[:], iwk[:])
            nc.vector.memset(ones_t[:], 1.0)
            nc.vector.memset(eps_t[:], EPS)
            if mtri is not None:
                nc.sync.dma_start(tri_t[:], mtri[:])
            nc.sync.dma_start(cos_t[:], cosT[:])
            nc.sync.dma_start(sin_t[:], sinT[:])

        def load_wo():
            for f in range(QH):
                nc.sync.dma_start(wo_t[:, f * H:(f + 1) * H],
                                  woT[f * HD:(f + 1) * HD, :])

        # norm+rope staged: s1 (right after the proj matmuls) does the
        # rotate-copy, square, and the cos-product -- the last PSUM read, so
        # the proj bank frees early; the var matmul (s2) trails by one
        # projection group; s3 finishes rstd + rope off the PSUM path.
        def norm_s1(pp, sc):
            sh = spool.tile([128, SC], F32, tag="sh", name="sh")
            nc.vector.tensor_copy(sh[0:64, :], pp[64:128, :])
            nc.vector.tensor_copy(sh[64:128, :], pp[0:64, :])
            # var from the rotated copy (rotation-invariant; iw_t rows are
            # pre-rotated host-side to match)
            sq = spool.tile([128, SC], MM, tag="sq", name="sq")
            nc.vector.tensor_mul(sq[:], sh[:], sh[:])
            uu = spool.tile([128, SC], F32, tag="uu", name="uu")
            nc.vector.tensor_mul(uu[:], pp[:], cos_t[:, sc * SC:(sc + 1) * SC])
            return sh, sq, uu

        def norm_s2(sq, iw_t):
            var = ps_mm.tile([1, SC], F32, tag="mm", name="var")
            nc.tensor.matmul(var[:], iw_t[:], sq[:], start=True, stop=True)
            return var

        def norm_s3(sh, uu, var, sc, hd, hat_dst):
            # sd = sqrt(var + eps)/qc via folded scale/bias; rs = qc/rms
            sd = spool.tile([1, SC], F32, tag="sd", name="sd")
            if hd is None:
                nc.scalar.activation(sd[:], var[:], AF.Sqrt, bias=eps_t[:])
            else:
                nc.scalar.activation(sd[:], var[:], AF.Sqrt,
                                     bias=qsb_t[:, hd:hd + 1],
                                     scale=qsc_t[:, hd:hd + 1])
            rs = spool.tile([1, SC], F32, tag="rs", name="rs")
            nc.vector.reciprocal_approx_fast(rs[:], sd[:])
            tt = spool.tile([128, SC], F32, tag="tt", name="tt")
            nc.vector.scalar_tensor_tensor(
                tt[:], sh[:], sgn_t[:], sin_t[:, sc * SC:(sc + 1) * SC],
                op0=OP.mult, op1=OP.mult)
            bb = spool.tile([128, SC], F32, tag="bb", name="bb")
            nc.gpsimd.partition_broadcast(bb[:], rs[:], 128)
            nc.vector.tensor_add(tt[:], tt[:], uu[:])
            nc.vector.tensor_mul(hat_dst, tt[:], bb[:])

        # ---- projections, per s-chunk -----------------------------------
        def hts_load(sc):
            hts = []
            for c in range(NHC):
                t = hpool.tile([128, SC], MM, tag="ht", name="ht")
                nc.sync.dma_start(t[:], hT[c * HC:(c + 1) * HC,
                                           sc * SC:(sc + 1) * SC])
                hts.append(t)
            return hts

        def proj_chunk(sc, hts):
            # five projections (k, q0..q3), staged so each var matmul is
            # emitted after the NEXT projection's matmul group
            specs = [(iwk_t, None, khat)] + [
                (iwq_t, hd, qhat[hd]) for hd in range(QH)]
            state = []  # (sh, sq, uu, spec)

            def do_mm(idx):
                pp = ps_mm.tile([128, SC], F32, tag="mm", name="pp")
                for c in range(NHC):
                    if idx == 0:
                        w_sl = wk_t[:, c * HD:(c + 1) * HD]
                    else:
                        base = c * QH * HD + (idx - 1) * HD
                        w_sl = wq_t[:, base:base + HD]
                    nc.tensor.matmul(pp[:], w_sl, hts[c][:],
                                     start=(c == 0), stop=(c == NHC - 1))
                sh, sq, uu = norm_s1(pp, sc)
                state.append((sh, sq, uu, specs[idx]))

            def finish_one():
                sh, sq, uu, (iw_t, hd, dst) = state.pop(0)
                var = norm_s2(sq, iw_t)
                norm_s3(sh, uu, var, sc, hd, dst[:, sc * SC:(sc + 1) * SC])

            do_mm(0)
            for idx in range(1, 5):
                do_mm(idx)
                finish_one()
            finish_one()
            # v-proj: natural [s, d] layout, N=128 matmuls
            for ss in range(4):
                vp = ps_v.tile([128, HD], F32, tag="v", name="vp")
                for c in range(NHC):
                    nc.tensor.matmul(vp[:], hts[c][:, ss * 128:(ss + 1) * 128],
                                     wv_t[:, c * HD:(c + 1) * HD],
                                     start=(c == 0), stop=(c == NHC - 1))
                col = (sc * 4 + ss) * 128
                nc.vector.tensor_copy(v_sb[:, col:col + 128], vp[:])

        # ---- attention + o_proj, per q-tile ------------------------------
        mask_starts = [sum(mask_counts[:i]) for i in range(NSC)]

        def attn_qtile(qi):
            mask_idx = mask_starts[qi]
            kts = [kt for kt in range(NKT) if plan[qi][kt] != "skip"]
            # load this q-tile's mask blocks (shared across heads)
            mtiles = {}
            for kt in kts:
                if plan[qi][kt] == "step":
                    mt = mpool.tile([128, SC], MM, tag="maskh", name="mh")
                    nc.sync.dma_start(mt[:], mhot[mask_idx])
                    mtiles[kt] = ("step", mt)
                    mask_idx += 1
                elif plan[qi][kt] == "mask":
                    mt = mpool.tile([128, SC], F32, tag="mask", name="mk")
                    nc.sync.dma_start(mt[:], mblk[mask_idx])
                    mtiles[kt] = ("mask", mt)
                    mask_idx += 1
            ats = []
            for hd in range(QH):
                qsl = qhat[hd][:, qi * SC:(qi + 1) * SC]
                pv = ps_pv.tile([128, SC], F32, tag="pv")
                es = ps_es.tile([1, SC], F32, tag="es")
                sts = {}
                # pipeline QK^T one k-tile ahead of exp/PV
                for j, kt in enumerate(kts):
                    st = ps_mm.tile([128, SC], F32, tag="mm")
                    kind, mt = mtiles.get(kt, (None, None))
                    nc.tensor.matmul(st[:], khat[:, kt * 128:(kt + 1) * 128],
                                     qsl, start=True, stop=(kind != "step"))
                    if kind == "step":
                        nc.tensor.matmul(st[:], tri_t[:], mt[:],
                                         start=False, stop=True)
                    elif kind == "mask":
                        nc.vector.tensor_add(st[:], st[:], mt[:])
                    sts[j] = st
                    if j >= 1:
                        _attn_tail(nc, j - 1, kts, sts, es, pv, v_sb,
                                   ones_t, MM, AF, epool)
                _attn_tail(nc, len(kts) - 1, kts, sts, es, pv, v_sb,
                           ones_t, MM, AF, epool)
                rs = spool.tile([1, SC], F32, tag="ars")
                nc.vector.reciprocal_approx_fast(rs[:], es[:])
                bb = spool.tile([128, SC], F32, tag="abb")
                nc.gpsimd.partition_broadcast(bb[:], rs[:], 128)
                at = atpool.tile([128, SC], MM, tag="at")
                nc.vector.tensor_mul(at[:], pv[:], bb[:])
                ats.append(at)
            # o_proj for this q-tile
            for ss in range(4):
                for ho in range(4):
                    op_t = ps_o.tile([128, SC], F32, tag="o")
                    for hd in range(QH):
                        nc.tensor.matmul(
                            op_t[:],
                            ats[hd][:, ss * 128:(ss + 1) * 128],
                            wo_t[:, hd * H + ho * SC:hd * H + (ho + 1) * SC],
                            start=(hd == 0), stop=(hd == QH - 1))
                    ob = opool_sb.tile([128, SC], F32, tag="osb")
                    nc.vector.tensor_copy(ob[:], op_t[:])
                    nc.sync.dma_start(
                        out[qi * SC + ss * 128:qi * SC + (ss + 1) * 128,
                            ho * SC:(ho + 1) * SC],
                        ob[:])

        # ---- driver: software-pipelined phase order ----------------------
        hts0 = hts_load(0)
        load_early()
        proj_chunk(0, hts0)
        hts1 = hts_load(1)
        proj_chunk(1, hts1)
        load_wo()
        attn_qtile(0)
        hts2 = hts_load(2)
        proj_chunk(2, hts2)
        attn_qtile(1)
        hts3 = hts_load(3)
        proj_chunk(3, hts3)
        attn_qtile(2)
        attn_qtile(3)


def _attn_tail(nc, j, kts, sts, es, pv, v_sb, ones_t, MM, AF, epool):
    """exp + PV + PSUM-accumulated denominator for pipelined k-tile j."""
    kt = kts[j]
    st = sts.pop(j)
    ex = epool.tile([128, SC], MM, tag="ex", name="ex")
    nc.scalar.activation(ex[:], st[:], AF.Exp)
    last = j == len(kts) - 1
    nc.tensor.matmul(pv[:], v_sb[:, kt * 128:(kt + 1) * 128], ex[:],
                     start=(j == 0), stop=last)
    nc.tensor.matmul(es[:], ones_t[:], ex[:], start=(j == 0), stop=last)


def _is_step(blk):
    """True if every batch/column is 0 for k < f and exactly -1e9 for k >= f."""
    isneg = blk == np.float32(-1e9)
    iszero = blk == 0.0
    if not (isneg | iszero).all():
        return False
    # per (b, q): suffix property along k
    f = isneg.argmax(axis=-1) + 0  # first masked k (0 if none masked)
    any_neg = isneg.any(axis=-1)
    kk = np.arange(blk.shape[-1])
    want = np.where(any_neg[..., None], kk[None, None] >= f[..., None], False)
    return bool((isneg == want).all())


def _mask_plan(mask):
    """Classify [qi][kt] blocks of the (q,k) mask, unified across batch."""
    plan = []
    for qi in range(NSC):
        row = []
        for kt in range(NKT):
            blk = mask[:, 0, qi * SC:(qi + 1) * SC, kt * KT:(kt + 1) * KT]
            if (blk <= SKIP_THRESH).all():
                row.append("skip")
            elif (blk == 0.0).all():
                row.append("zero")
            elif _is_step(blk):
                row.append("step")
            else:
                row.append("mask")
        # guard: a q-tile with no included block would divide by zero
        if all(s == "skip" for s in row):
            row[0] = "mask"
        plan.append(row)
    return plan


def kernel(hidden_states, cos, sin, attention_mask, wq, wk, wv, wo,
           q_norm_w, k_norm_w, ssmax_scale):
    global LAST_EXEC_NS
    import os
    import ml_dtypes
    from concourse.bass_utils import run_bass_kernel_spmd

    f32 = np.float32
    hidden_states = np.asarray(hidden_states, f32)
    cos = np.asarray(cos, f32)
    sin = np.asarray(sin, f32)
    attention_mask = np.asarray(attention_mask, f32)
    wq = np.asarray(wq, f32)
    wk = np.asarray(wk, f32)
    wv = np.asarray(wv, f32)
    wo = np.asarray(wo, f32)
    q_norm_w = np.asarray(q_norm_w, f32)
    k_norm_w = np.asarray(k_norm_w, f32)
    ssmax = np.asarray(ssmax_scale, f32).reshape(NH)

    plan = _mask_plan(attention_mask)
    mask_counts = [sum(1 for s in row if s in ("mask", "step")) for row in plan]
    key = (tuple(tuple(r) for r in plan),)
    if key not in _compiled_cache:
        _compiled_cache[key] = _build_program(plan, mask_counts)
    nc = _compiled_cache[key]

    bf16 = ml_dtypes.bfloat16
    qw = np.tile(q_norm_w, QH)
    sgn_np = np.concatenate([-np.ones(64, f32), np.ones(64, f32)])[:, None]
    iwq_np = np.roll(1.0 / (HD * q_norm_w ** 2), -64).astype(bf16)[:, None]
    iwk_np = np.roll(1.0 / (HD * k_norm_w ** 2), -64).astype(bf16)[:, None]
    cosT_np = np.ascontiguousarray(cos.T)
    sinT_np = np.ascontiguousarray(sin.T)

    in_maps = []
    for core in range(NCORES):
        b, g = divmod(core, TP)
        hTm = np.ascontiguousarray(hidden_states[b].T).astype(bf16)
        wq_s = wq[g * QH * HD:(g + 1) * QH * HD] * qw[:, None]
        wk_s = wk[g * HD:(g + 1) * HD] * k_norm_w[:, None]
        wv_s = wv[g * HD:(g + 1) * HD]
        wo_s = wo[:, g * QH * HD:(g + 1) * QH * HD]
        qcv = np.array([ssmax[g * QH + i] * math.log(S) / math.sqrt(HD)
                        for i in range(QH)], f32)
        qsc_np = (1.0 / qcv ** 2)[None, :].astype(f32)
        qsb_np = (EPS / qcv ** 2)[None, :].astype(f32)
        m = {
            "hT": hTm,
            "wqT": np.ascontiguousarray(wq_s.T).astype(bf16),
            "wkT": np.ascontiguousarray(wk_s.T).astype(bf16),
            "wvT": np.ascontiguousarray(wv_s.T).astype(bf16),
            "woT": np.ascontiguousarray(wo_s.T).astype(bf16),
            "cosT": cosT_np, "sinT": sinT_np, "sgn": sgn_np,
            "qsc": qsc_np, "qsb": qsb_np, "iwq": iwq_np, "iwk": iwk_np,
        }
        n_mask = sum(mask_counts)
        if n_mask:
            blocks = np.zeros((n_mask, KT, SC), f32)
            hots = np.zeros((n_mask, KT, SC), f32)
            i = 0
            for qi in range(NSC):
                for kt in range(NKT):
                    kind = plan[qi][kt]
                    if kind not in ("mask", "step"):
                        continue
                    blkT = attention_mask[
                        b, 0, qi * SC:(qi + 1) * SC,
                        kt * KT:(kt + 1) * KT].T
                    if kind == "mask":
                        blocks[i] = blkT
                    else:
                        isneg = blkT == np.float32(-1e9)
                        f = isneg.argmax(axis=0)
                        anyneg = isneg.any(axis=0)
                        qsel = np.nonzero(anyneg)[0]
                        hots[i][f[qsel], qsel] = 1.0
                    i += 1
            m["mblk"] = blocks
            m["mhot"] = hots.astype(bf16)
            # tri[r, k] = -1e9 * [k >= r]; lhsT layout [r(part), k(free)]
            tri = (-1e9 * (np.arange(KT)[None, :] >= np.arange(KT)[:, None]))
            m["mtri"] = np.ascontiguousarray(tri).astype(bf16)
        in_maps.append(m)

    trace = bool(int(os.environ.get("BASS_KERNEL_TRACE", "0")))
    res = run_bass_kernel_spmd(nc, in_maps, list(range(NCORES)), trace=trace)
    LAST_EXEC_NS = res.exec_time_ns
    globals()["LAST_RESULTS"] = res

    final = np.zeros((B, S, H), f32)
    for core in range(NCORES):
        b = core // TP
        final[b] += res.results[core]["out"]
    return final
